# revision 18
# baseline (speedup 1.0000x reference)
"""MoE gate (group-limited top-k routing) as a Bass/Tile kernel for 8 TRN2 cores.

Computes, per token:
  logits = hidden @ W            (K=7168, E=256)
  scores = sigmoid(logits) + bias
  group-limited routing: top-2-sum per group of 32 -> top-4 groups of 8
  top-8 of masked scores, renormalized, * 2.5

Sharding: data-parallel over tokens (1024 tokens/core), W + bias replicated.

Matmul schemes:
  f32r1 (default): weights and transposed hidden chunks fed to the PE as
    float32r (1 cyc/row when the moving dim is >= 256, vs 4 for fp32;
    transposes 1.5 cyc/row vs 2). No cast passes at all: DRAM fp32 bits are
    DMA'd straight into fp32 tiles and bitcast to f32r at the PE ops.
    Accuracy is fp32-class (f32r mantissa ~19 bits).
  f16x1: x and W rounded to fp16 (cast fused into the PSUM copyback /
    weight-prep), fp32 transposes, single fp16 matmul per K-chunk.
    Logit error ~1e-3 relative.

Schedule (both schemes):
  - weights: 7 independent DMAs into a resident fp32 buffer, dispatched on
    the gpsimd queue so the sync queue starts streaming hidden tiles at t=0.
  - per token tile: 7 batches of 8 PE transposes -> PSUM; copyback to SBUF
    on scalar (5/7) or vector (2/7); matmul batch b runs two transpose
    batches behind so copybacks never stall the PE.
  - the routing epilogue for tile t-1 (sigmoid on scalar, rest on vector)
    is emitted inside tile t's batch loop so it cannot head-of-line block
    tile t's copybacks on those engines.
"""

import sys

if "/opt/trn_rl_repo" not in sys.path:
    sys.path.insert(0, "/opt/trn_rl_repo")

import numpy as np

import concourse.bacc as bacc
import concourse.bass as bass
import concourse.mybir as mybir
import concourse.tile as tile
from concourse import bass_utils
from concourse.masks import make_identity

P = 128
TOP_K = 8
N_GROUP = 8
TOPK_GROUP = 4
SCALE = 2.5

N_CORES = 8
TOKENS = 8192
HIDDEN = 7168
EXPERTS = 256


def build_moe_gate(
    tokens_per_core=TOKENS // N_CORES,
    hidden=HIDDEN,
    n_experts=EXPERTS,
    scheme="f16x1",
):
    KC = hidden // P          # K-chunks of 128
    TT = tokens_per_core // P  # token tiles of 128
    GS = n_experts // N_GROUP  # experts per group
    BATCH = 8 if KC % 8 == 0 else 4   # transposes batched per PSUM copyback
    WB = 8 if KC % 8 == 0 else 4      # weight chunks per DMA slice
    NB = KC // BATCH
    DEPTH = 5                  # matmul batches run this far behind transposes
    f32 = mybir.dt.float32
    f16 = mybir.dt.float16
    f32r = mybir.dt.float32r

    nc = bacc.Bacc("TRN2", target_bir_lowering=False, debug=False)
    hs = nc.dram_tensor(
        "hidden_states", [tokens_per_core, hidden], f32, kind="ExternalInput"
    ).ap()
    wk = nc.dram_tensor("kernel", [hidden, n_experts], f32, kind="ExternalInput").ap()
    bias = nc.dram_tensor(
        "e_score_correction_bias", [n_experts], f32, kind="ExternalInput"
    ).ap()
    out = nc.dram_tensor(
        "topk_out", [tokens_per_core, TOP_K], f32, kind="ExternalOutput"
    ).ap()

    # which engine copies each transpose batch out of PSUM
    CB_ENG = {0: "act", 1: "dve", 2: "act", 3: "act", 4: "act", 5: "dve", 6: "act"}

    with tile.TileContext(nc) as tc:
        with (
            tc.tile_pool(name="const", bufs=1) as cpool,
            tc.tile_pool(name="hload", bufs=5) as hpool,
            tc.tile_pool(name="ht", bufs=7) as htpool,
            tc.tile_pool(name="ptr", bufs=3, space="PSUM") as ptpool,
            tc.tile_pool(name="plog", bufs=2, space="PSUM") as plpool,
            tc.tile_pool(name="route", bufs=2) as rpool,
        ):
            # identity must exist before the first transpose: build it before
            # anything else is queued on gpsimd
            identity = cpool.tile([P, P], f32)
            make_identity(nc, identity)
            id_t = identity.bitcast(f32r) if scheme == "f32r1" else identity

            # resident replicated weights, cast to fp16 IN the DMA: gpsimd
            # (software DGE) supports casting DMAs, so the fp32 DRAM weights
            # land directly as fp16 in SBUF -- no cast pass on any compute
            # engine, usable the moment each slice arrives.
            WSL = WB // 2
            NWS = KC // WSL
            wh = cpool.tile([P, KC, n_experts], f16)
            wk_view = wk.rearrange("(kc p) e -> p kc e", p=P)

            def w_dma(wb):
                ws = slice(wb * WSL, (wb + 1) * WSL)
                nc.gpsimd.dma_start(out=wh[:, ws, :], in_=wk_view[:, ws, :])

            for wb in range(NWS):
                w_dma(wb)

            bias_sb = cpool.tile([P, n_experts], f32)
            bias_bcast = bass.AP(
                tensor=bias.tensor, offset=bias.offset, ap=[[0, P]] + list(bias.ap)
            )
            nc.gpsimd.dma_start(out=bias_sb, in_=bias_bcast)

            def make_epilogue(t, logits_ps):
                def ep(phase):
                    if phase == "sig":
                        sc = rpool.tile([P, n_experts], f32)
                        nc.scalar.activation(
                            sc, logits_ps, mybir.ActivationFunctionType.Sigmoid
                        )
                        ep.sc = sc
                        return
                    sc = ep.sc
                    nc.vector.tensor_add(sc, sc, bias_sb)
                    # top-2 sum per group of GS experts
                    m8 = rpool.tile([P, N_GROUP * 8], f32)
                    for g in range(N_GROUP):
                        nc.vector.max(
                            m8[:, g * 8 : (g + 1) * 8],
                            sc[:, g * GS : (g + 1) * GS],
                        )
                    m8v = m8.rearrange("p (g k) -> p g k", k=8)
                    gsum = rpool.tile([P, N_GROUP], f32)
                    nc.vector.tensor_add(gsum, m8v[:, :, 0], m8v[:, :, 1])
                    # top-TOPK_GROUP groups -> 0/1 mask via threshold
                    gmax = rpool.tile([P, 8], f32)
                    nc.vector.max(gmax, gsum)
                    gmask = rpool.tile([P, N_GROUP], f32)
                    nc.vector.tensor_scalar(
                        gmask,
                        gsum,
                        gmax[:, TOPK_GROUP - 1 : TOPK_GROUP],
                        None,
                        op0=mybir.AluOpType.is_ge,
                    )
                    masked = rpool.tile([P, n_experts], f32)
                    nc.vector.tensor_mul(
                        masked.rearrange("p (g e) -> p g e", g=N_GROUP),
                        sc.rearrange("p (g e) -> p g e", g=N_GROUP),
                        gmask[:, :, None].broadcast_to([P, N_GROUP, GS]),
                    )
                    top8 = rpool.tile([P, TOP_K], f32)
                    nc.vector.max(top8, masked)
                    dsum = rpool.tile([P, 1], f32)
                    nc.vector.reduce_sum(dsum, top8, axis=mybir.AxisListType.X)
                    rcp = rpool.tile([P, 1], f32)
                    nc.vector.reciprocal(rcp, dsum)
                    wout = rpool.tile([P, TOP_K], f32)
                    nc.vector.tensor_scalar(
                        wout,
                        top8,
                        rcp,
                        SCALE,
                        op0=mybir.AluOpType.mult,
                        op1=mybir.AluOpType.mult,
                    )
                    nc.sync.dma_start(out=out[t * P : (t + 1) * P, :], in_=wout)

                return ep

            prev_ep = None
            for t in range(TT):
                htile = hpool.tile([P, hidden], f32)
                for l in range(NB):
                    sl = slice(l * BATCH * P, (l + 1) * BATCH * P)
                    # split slice dispatches across two queues for deeper
                    # DMA in-flight depth (gpsimd is idle mid-run)
                    eng = nc.sync if l % 2 == 0 else nc.gpsimd
                    eng.dma_start(
                        out=htile[:, sl], in_=hs[t * P : (t + 1) * P, sl]
                    )


                logits_ps = plpool.tile([P, n_experts], f32)
                hTs = [None] * NB

                def mm_batch(b):
                    for j in range(BATCH):
                        k = b * BATCH + j
                        if scheme == "f32r1":
                            lhsT = hTs[b][:, j * P : (j + 1) * P].bitcast(f32r)
                            rhs = wk32[:, k, :].bitcast(f32r)
                        else:
                            lhsT = hTs[b][:, j * P : (j + 1) * P]
                            rhs = wh[:, k, :]
                        nc.tensor.matmul(
                            logits_ps,
                            lhsT=lhsT,
                            rhs=rhs,
                            start=(k == 0),
                            stop=(k == KC - 1),
                        )

                for b in range(NB):
                    tp = ptpool.tile([P, BATCH * P], f32)
                    for j in range(BATCH):
                        k = b * BATCH + j
                        src = htile[:, k * P : (k + 1) * P]
                        dst = tp[:, j * P : (j + 1) * P]
                        if scheme == "f32r1":
                            nc.tensor.transpose(
                                dst.bitcast(f32r), src.bitcast(f32r), id_t
                            )
                        else:
                            nc.tensor.transpose(dst, src, id_t)
                    # PSUM -> SBUF copyback (f16x1: doubles as the fp16 cast)
                    hT = htpool.tile(
                        [P, BATCH * P], f16 if scheme == "f16x1" else f32
                    )
                    if CB_ENG[b] == "act":
                        nc.scalar.activation(
                            hT, tp, mybir.ActivationFunctionType.Copy
                        )
                    else:
                        nc.vector.tensor_copy(hT, tp)
                    hTs[b] = hT
                    # interleave previous tile's epilogue so it can't block
                    # this tile's copybacks behind it in the engine queues
                    if prev_ep is not None:
                        if b == 0:
                            prev_ep("sig")
                        elif b == 1:
                            prev_ep("route")
                    if b >= DEPTH:
                        mm_batch(b - DEPTH)
                for b in range(NB - DEPTH, NB):
                    mm_batch(b)

                prev_ep = make_epilogue(t, logits_ps)
            prev_ep("sig")
            prev_ep("route")

    nc.compile()
    return nc


_CACHE = {}


def _built_nc():
    if "nc" not in _CACHE:
        _CACHE["nc"] = build_moe_gate()
    return _CACHE["nc"]


def kernel(hidden_states, kernel, e_score_correction_bias):
    hs = np.ascontiguousarray(np.asarray(hidden_states), dtype=np.float32)
    wk = np.ascontiguousarray(np.asarray(kernel), dtype=np.float32)
    bi = np.ascontiguousarray(np.asarray(e_score_correction_bias), dtype=np.float32)
    assert hs.shape == (TOKENS, HIDDEN) and wk.shape == (HIDDEN, EXPERTS)

    tpc = TOKENS // N_CORES
    nc = _built_nc()
    in_maps = [
        {
            "hidden_states": hs[i * tpc : (i + 1) * tpc],
            "kernel": wk,
            "e_score_correction_bias": bi,
        }
        for i in range(N_CORES)
    ]
    res = bass_utils.run_bass_kernel_spmd(nc, in_maps, core_ids=list(range(N_CORES)))
    return np.concatenate(
        [res.results[i]["topk_out"] for i in range(N_CORES)], axis=0
    )


# revision 19
# speedup vs baseline: 1.0838x; 1.0838x over previous
"""MoE gate (group-limited top-k routing) as a Bass/Tile kernel for 8 TRN2 cores.

Computes, per token:
  logits = hidden @ W            (K=7168, E=256)
  scores = sigmoid(logits) + bias
  group-limited routing: top-2-sum per group of 32 -> top-4 groups of 8
  top-8 of masked scores, renormalized, * 2.5

Sharding: data-parallel over tokens (1024 tokens/core), W + bias replicated.

Matmul schemes:
  f32r1 (default): weights and transposed hidden chunks fed to the PE as
    float32r (1 cyc/row when the moving dim is >= 256, vs 4 for fp32;
    transposes 1.5 cyc/row vs 2). No cast passes at all: DRAM fp32 bits are
    DMA'd straight into fp32 tiles and bitcast to f32r at the PE ops.
    Accuracy is fp32-class (f32r mantissa ~19 bits).
  f16x1: x and W rounded to fp16 (cast fused into the PSUM copyback /
    weight-prep), fp32 transposes, single fp16 matmul per K-chunk.
    Logit error ~1e-3 relative.

Schedule (both schemes):
  - weights: 7 independent DMAs into a resident fp32 buffer, dispatched on
    the gpsimd queue so the sync queue starts streaming hidden tiles at t=0.
  - per token tile: 7 batches of 8 PE transposes -> PSUM; copyback to SBUF
    on scalar (5/7) or vector (2/7); matmul batch b runs two transpose
    batches behind so copybacks never stall the PE.
  - the routing epilogue for tile t-1 (sigmoid on scalar, rest on vector)
    is emitted inside tile t's batch loop so it cannot head-of-line block
    tile t's copybacks on those engines.
"""

import sys

if "/opt/trn_rl_repo" not in sys.path:
    sys.path.insert(0, "/opt/trn_rl_repo")

import numpy as np

import concourse.bacc as bacc
import concourse.bass as bass
import concourse.mybir as mybir
import concourse.tile as tile
from concourse import bass_utils
from concourse.masks import make_identity

P = 128
TOP_K = 8
N_GROUP = 8
TOPK_GROUP = 4
SCALE = 2.5

N_CORES = 8
TOKENS = 8192
HIDDEN = 7168
EXPERTS = 256


def build_moe_gate(
    tokens_per_core=TOKENS // N_CORES,
    hidden=HIDDEN,
    n_experts=EXPERTS,
    scheme="f16x1",
):
    KC = hidden // P          # K-chunks of 128
    TT = tokens_per_core // P  # token tiles of 128
    GS = n_experts // N_GROUP  # experts per group
    BATCH = 8 if KC % 8 == 0 else 4   # transposes batched per PSUM copyback
    WB = 8 if KC % 8 == 0 else 4      # weight chunks per DMA slice
    NB = KC // BATCH
    DEPTH = 3                  # matmul batches run this far behind transposes
    f32 = mybir.dt.float32
    f16 = mybir.dt.float16
    f32r = mybir.dt.float32r

    nc = bacc.Bacc("TRN2", target_bir_lowering=False, debug=False)
    hs = nc.dram_tensor(
        "hidden_states", [tokens_per_core, hidden], f32, kind="ExternalInput"
    ).ap()
    wk = nc.dram_tensor("kernel", [hidden, n_experts], f32, kind="ExternalInput").ap()
    bias = nc.dram_tensor(
        "e_score_correction_bias", [n_experts], f32, kind="ExternalInput"
    ).ap()
    out = nc.dram_tensor(
        "topk_out", [tokens_per_core, TOP_K], f32, kind="ExternalOutput"
    ).ap()

    # which engine copies each transpose batch out of PSUM
    CB_ENG = {0: "act", 1: "dve", 2: "act", 3: "act", 4: "act", 5: "dve", 6: "act"}

    with tile.TileContext(nc) as tc:
        with (
            tc.tile_pool(name="const", bufs=1) as cpool,
            tc.tile_pool(name="hload", bufs=3) as hpool,
            tc.tile_pool(name="ht", bufs=5) as htpool,
            tc.tile_pool(name="ptr", bufs=3, space="PSUM") as ptpool,
            tc.tile_pool(name="plog", bufs=2, space="PSUM") as plpool,
            tc.tile_pool(name="route", bufs=2) as rpool,
        ):
            # identity must exist before the first transpose: build it before
            # anything else is queued on gpsimd
            identity = cpool.tile([P, P], f32)
            make_identity(nc, identity)
            id_t = identity.bitcast(f32r) if scheme == "f32r1" else identity

            # resident replicated weights in fine 4-chunk slices. DMAs split
            # between the gpsimd queue and the sync queue (interleaved with
            # tile-0's hidden slices, below) so the scalar/vector engines
            # stay clean for copybacks. fp16 casts: first 4 slices on the
            # vector engine (needed within ~8us), the rest on gpsimd.
            WSL = WB // 2
            NWS = KC // WSL
            wk32 = cpool.tile([P, KC, n_experts], f32)
            wh = cpool.tile([P, KC, n_experts], f16)
            wk_view = wk.rearrange("(kc p) e -> p kc e", p=P)

            def w_dma(wb, eng):
                ws = slice(wb * WSL, (wb + 1) * WSL)
                eng.dma_start(out=wk32[:, ws, :], in_=wk_view[:, ws, :])

            def w_cast(wb, eng):
                ws = slice(wb * WSL, (wb + 1) * WSL)
                eng.tensor_copy(wh[:, ws, :], wk32[:, ws, :])

            for wb in range(1, NWS, 2):
                w_dma(wb, nc.gpsimd)

            bias_sb = cpool.tile([P, n_experts], f32)
            bias_bcast = bass.AP(
                tensor=bias.tensor, offset=bias.offset, ap=[[0, P]] + list(bias.ap)
            )
            nc.gpsimd.dma_start(out=bias_sb, in_=bias_bcast)

            def make_epilogue(t, logits_ps):
                def ep(phase):
                    if phase == "sig":
                        sc = rpool.tile([P, n_experts], f32)
                        nc.scalar.activation(
                            sc, logits_ps, mybir.ActivationFunctionType.Sigmoid
                        )
                        ep.sc = sc
                        return
                    sc = ep.sc
                    nc.vector.tensor_add(sc, sc, bias_sb)
                    # top-2 sum per group of GS experts
                    m8 = rpool.tile([P, N_GROUP * 8], f32)
                    for g in range(N_GROUP):
                        nc.vector.max(
                            m8[:, g * 8 : (g + 1) * 8],
                            sc[:, g * GS : (g + 1) * GS],
                        )
                    m8v = m8.rearrange("p (g k) -> p g k", k=8)
                    gsum = rpool.tile([P, N_GROUP], f32)
                    nc.vector.tensor_add(gsum, m8v[:, :, 0], m8v[:, :, 1])
                    # top-TOPK_GROUP groups -> 0/1 mask via threshold
                    gmax = rpool.tile([P, 8], f32)
                    nc.vector.max(gmax, gsum)
                    gmask = rpool.tile([P, N_GROUP], f32)
                    nc.vector.tensor_scalar(
                        gmask,
                        gsum,
                        gmax[:, TOPK_GROUP - 1 : TOPK_GROUP],
                        None,
                        op0=mybir.AluOpType.is_ge,
                    )
                    masked = rpool.tile([P, n_experts], f32)
                    nc.vector.tensor_mul(
                        masked.rearrange("p (g e) -> p g e", g=N_GROUP),
                        sc.rearrange("p (g e) -> p g e", g=N_GROUP),
                        gmask[:, :, None].broadcast_to([P, N_GROUP, GS]),
                    )
                    top8 = rpool.tile([P, TOP_K], f32)
                    nc.vector.max(top8, masked)
                    dsum = rpool.tile([P, 1], f32)
                    nc.vector.reduce_sum(dsum, top8, axis=mybir.AxisListType.X)
                    rcp = rpool.tile([P, 1], f32)
                    nc.vector.reciprocal(rcp, dsum)
                    wout = rpool.tile([P, TOP_K], f32)
                    nc.vector.tensor_scalar(
                        wout,
                        top8,
                        rcp,
                        SCALE,
                        op0=mybir.AluOpType.mult,
                        op1=mybir.AluOpType.mult,
                    )
                    nc.sync.dma_start(out=out[t * P : (t + 1) * P, :], in_=wout)

                return ep

            prev_ep = None
            for t in range(TT):
                htile = hpool.tile([P, hidden], f32)
                for l in range(NB):
                    sl = slice(l * BATCH * P, (l + 1) * BATCH * P)
                    nc.sync.dma_start(
                        out=htile[:, sl], in_=hs[t * P : (t + 1) * P, sl]
                    )
                    if t == 0:
                        # interleave the even weight-slice DMAs between
                        # tile-0's hidden slices on the sync queue
                        if l >= 1 and 2 * (l - 1) < NWS:
                            w_dma(2 * (l - 1), nc.sync)
                        if l == NB - 1:
                            for wb in range(2 * (NB - 1), NWS, 2):
                                w_dma(wb, nc.sync)


                logits_ps = plpool.tile([P, n_experts], f32)
                hTs = [None] * NB

                def mm_batch(b):
                    for j in range(BATCH):
                        k = b * BATCH + j
                        if scheme == "f32r1":
                            lhsT = hTs[b][:, j * P : (j + 1) * P].bitcast(f32r)
                            rhs = wk32[:, k, :].bitcast(f32r)
                        else:
                            lhsT = hTs[b][:, j * P : (j + 1) * P]
                            rhs = wh[:, k, :]
                        nc.tensor.matmul(
                            logits_ps,
                            lhsT=lhsT,
                            rhs=rhs,
                            start=(k == 0),
                            stop=(k == KC - 1),
                        )

                for b in range(NB):
                    tp = ptpool.tile([P, BATCH * P], f32)
                    for j in range(BATCH):
                        k = b * BATCH + j
                        src = htile[:, k * P : (k + 1) * P]
                        dst = tp[:, j * P : (j + 1) * P]
                        if scheme == "f32r1":
                            nc.tensor.transpose(
                                dst.bitcast(f32r), src.bitcast(f32r), id_t
                            )
                        else:
                            nc.tensor.transpose(dst, src, id_t)
                    # PSUM -> SBUF copyback (f16x1: doubles as the fp16 cast)
                    hT = htpool.tile(
                        [P, BATCH * P], f16 if scheme == "f16x1" else f32
                    )
                    if CB_ENG[b] == "act":
                        nc.scalar.activation(
                            hT, tp, mybir.ActivationFunctionType.Copy
                        )
                    else:
                        nc.vector.tensor_copy(hT, tp)
                    hTs[b] = hT
                    # interleave previous tile's epilogue so it can't block
                    # this tile's copybacks behind it in the engine queues
                    if prev_ep is not None:
                        if b == 0:
                            prev_ep("sig")
                        elif b == 1:
                            prev_ep("route")
                    if t == 0 and b < 2:
                        # early weight-slice casts on the vector engine,
                        # right after its tile-0 copyback duties
                        w_cast(2 * b, nc.vector)
                        w_cast(2 * b + 1, nc.vector)
                        if b == 1:
                            # remaining casts on gpsimd. Must be emitted
                            # AFTER every w_dma above (the dep tracker goes
                            # by program order; earlier emission would read
                            # uninitialized wk32)
                            for wb in range(4, NWS):
                                w_cast(wb, nc.gpsimd)
                    if b >= DEPTH:
                        mm_batch(b - DEPTH)
                for b in range(NB - DEPTH, NB):
                    mm_batch(b)

                prev_ep = make_epilogue(t, logits_ps)
            prev_ep("sig")
            prev_ep("route")

    nc.compile()
    return nc


_CACHE = {}


def _built_nc():
    if "nc" not in _CACHE:
        _CACHE["nc"] = build_moe_gate()
    return _CACHE["nc"]


def kernel(hidden_states, kernel, e_score_correction_bias):
    hs = np.ascontiguousarray(np.asarray(hidden_states), dtype=np.float32)
    wk = np.ascontiguousarray(np.asarray(kernel), dtype=np.float32)
    bi = np.ascontiguousarray(np.asarray(e_score_correction_bias), dtype=np.float32)
    assert hs.shape == (TOKENS, HIDDEN) and wk.shape == (HIDDEN, EXPERTS)

    tpc = TOKENS // N_CORES
    nc = _built_nc()
    in_maps = [
        {
            "hidden_states": hs[i * tpc : (i + 1) * tpc],
            "kernel": wk,
            "e_score_correction_bias": bi,
        }
        for i in range(N_CORES)
    ]
    res = bass_utils.run_bass_kernel_spmd(nc, in_maps, core_ids=list(range(N_CORES)))
    return np.concatenate(
        [res.results[i]["topk_out"] for i in range(N_CORES)], axis=0
    )
